# revision 1
# baseline (speedup 1.0000x reference)
"""AttentionalPropagation kernel for Trainium2 (Bass/Tile), 8-core SPMD.

x: [B=64, C=512, L=4096] f32.  Per location l: self-attention over the B axis
(q=k=v, head dim C), out = x + msg.  Sharded over L: each of 8 cores handles
L/8 = 512 locations.

Per-core dataflow (block of LB=64 locations, pairs of 2 locations packed to
fill the 128-wide PE array):
  - DMA x in "qT" layout: one [c=128, ci=4, b=64, l=LB] f32 tile (256B runs)
  - cast f32 -> bf16 with (b,l)->(l,b) permute on GPSIMD (idle engine), so
    each location-pair is a contiguous 128-column matmul operand
  - mm1: scores pair [128,128] = qT^T @ qT accumulated over 4 C-chunks (PE)
  - exp(scores/sqrt(C)) on the two diagonal 64x64 blocks into block-diag E
    (off-diagonal zeros persist in 4 rotating E tiles, zeroed once) (ACT)
  - rowsum over full E rows -> reciprocal (DVE, zeros harmless)
  - q_BC: 4 identity-matmul transposes -> [128(j,b), 512(c)] psum (PE),
    copy-cast to bf16 sbuf (DVE)
  - mm2: msg pair [128,512] = E(block-diag) @ q_BC, one K=128 N=512 mm (PE)
  - copy psum->sbuf bf16 with per-partition scale = 1/rowsum (ACT)
  - 4 identity-matmul transposes back to [c, (j,b)] psum (PE), one merged
    DVE add into the f32 x tile in place, DMA the updated tile out
"""

import numpy as np

B, C, L_FULL, N_CORES = 64, 512, 4096, 8
LS = L_FULL // N_CORES  # 512 locations per core
LB = 64                 # locations per block
N_BLK = LS // LB        # 8
N_PAIR = LB // 2        # 32 pairs per block
CCH = C // 128          # 4 c-chunks
SCALE = 1.0 / float(C) ** 0.5


def build_nc():
    from contextlib import ExitStack

    import concourse.bass as bass
    import concourse.mybir as mybir
    from concourse.masks import make_identity
    from concourse.tile import TileContext

    f32 = mybir.dt.float32
    bf16 = mybir.dt.bfloat16
    AF = mybir.ActivationFunctionType

    nc = bass.Bass()
    x = nc.dram_tensor("x", [B, C, LS], f32, kind="ExternalInput")
    y = nc.dram_tensor("y", [B, C, LS], f32, kind="ExternalOutput")

    with ExitStack() as ctx:
        tc = ctx.enter_context(TileContext(nc))
        const = ctx.enter_context(tc.tile_pool(name="const", bufs=1))
        xt_pool = ctx.enter_context(tc.tile_pool(name="xt", bufs=2))
        qt_pool = ctx.enter_context(tc.tile_pool(name="qt", bufs=2))
        sm_pool = ctx.enter_context(tc.tile_pool(name="sm", bufs=6))
        ps_s_pool = ctx.enter_context(tc.tile_pool(name="ps_s", bufs=2, space="PSUM"))
        ps_t_pool = ctx.enter_context(tc.tile_pool(name="ps_t", bufs=2, space="PSUM"))
        ps_m_pool = ctx.enter_context(tc.tile_pool(name="ps_m", bufs=2, space="PSUM"))
        ps_o_pool = ctx.enter_context(tc.tile_pool(name="ps_o", bufs=2, space="PSUM"))

        ident = const.tile([128, 128], bf16)
        make_identity(nc, ident)

        for blk in range(N_BLK):
            l0 = blk * LB
            # one big f32 tile per block: [c, ci, b, l]; doubles as out staging
            xt_t = xt_pool.tile([128, CCH, B, LB], f32, name="xt", tag="xt")
            qt = []
            for ci in range(CCH):
                nc.sync.dma_start(
                    out=xt_t[:, ci],
                    in_=x[:, ci * 128 : (ci + 1) * 128, l0 : l0 + LB].rearrange(
                        "b c l -> c b l"
                    ),
                )
                qt_t = qt_pool.tile([128, LB * B], bf16, name=f"qt{ci}", tag=f"qt{ci}")
                # cast + relayout (b, l) -> (l, b) so each location pair is a
                # contiguous 128-column slice for matmul operands; spread
                # across gpsimd/DVE/ACT to hedge unmodeled strided-AP costs
                qdst = qt_t.rearrange("c (l b) -> c l b", b=B)
                qsrc = xt_t[:, ci].rearrange("c b l -> c l b")
                if ci < 2:
                    nc.gpsimd.tensor_copy(qdst, qsrc)
                elif ci == 2:
                    nc.vector.tensor_copy(qdst, qsrc)
                else:
                    nc.scalar.activation(qdst, qsrc, AF.Copy)
                qt.append(qt_t)

            for p in range(N_PAIR):
                # contiguous columns [p*128, (p+1)*128) = (j outer, b inner):
                # matmul M/N index = j*64+b  (pair-stacked)
                def pairT(tiles, ci, p=p):
                    return tiles[ci][:, p * 128 : (p + 1) * 128]

                # mm1: scores for the pair (plus ignored cross blocks)
                ps_s = ps_s_pool.tile([128, 128], f32)
                for ci in range(CCH):
                    nc.tensor.matmul(
                        ps_s,
                        pairT(qt, ci),
                        pairT(qt, ci),
                        start=(ci == 0),
                        stop=(ci == CCH - 1),
                    )

                # one full-tile exp; cross blocks are garbage but never read
                e = sm_pool.tile([128, 128], bf16, tag="e")
                nc.scalar.activation(e, ps_s, AF.Exp, scale=SCALE)

                # rowsums over the diagonal blocks only (per partition half)
                rs = sm_pool.tile([128, 1], f32, tag="rs")
                nc.vector.reduce_sum(rs[0:64], e[0:64, 0:64], axis=mybir.AxisListType.X)
                nc.vector.reduce_sum(
                    rs[64:128], e[64:128, 64:128], axis=mybir.AxisListType.X
                )
                inv = sm_pool.tile([128, 1], f32, tag="inv")
                nc.vector.reciprocal(inv, rs)

                # q_BC: transpose each qT chunk via identity matmul
                ps_t = ps_t_pool.tile([128, 512], f32)
                for ci in range(CCH):
                    nc.tensor.matmul(
                        ps_t[:, ci * 128 : (ci + 1) * 128],
                        pairT(qt, ci),
                        ident,
                        start=True,
                        stop=True,
                    )
                qbc = sm_pool.tile([128, 512], bf16, tag="qbc")
                if p % 2 == 0:
                    nc.vector.tensor_copy(qbc, ps_t)
                else:
                    nc.scalar.activation(qbc, ps_t, AF.Copy)

                # mm2: two row/col-tiled K=64 matmuls, one per location
                ps_m = ps_m_pool.tile([128, 512], f32)
                nc.tensor.matmul(
                    ps_m[0:64, :], e[0:64, 0:64], qbc[0:64, :],
                    start=True, stop=True, tile_position=(0, 0),
                )
                nc.tensor.matmul(
                    ps_m[64:128, :], e[64:128, 64:128], qbc[64:128, :],
                    start=True, stop=True, tile_position=(64, 64),
                )

                # scale rows by 1/rowsum while copying out of PSUM
                msg = sm_pool.tile([128, 512], bf16, tag="msg")
                nc.scalar.activation(msg, ps_m, AF.Copy, scale=inv)

                # transpose back to [c, (j, b)] and add into x tile (f32)
                ps_o = ps_o_pool.tile([128, CCH * 128], f32)
                for ci in range(CCH):
                    nc.tensor.matmul(
                        ps_o[:, ci * 128 : (ci + 1) * 128],
                        msg[:, ci * 128 : (ci + 1) * 128],
                        ident,
                        start=True,
                        stop=True,
                    )
                dst = xt_t[:, :, :, 2 * p : 2 * p + 2]  # [128, ci, b, j]
                src = ps_o.rearrange("c (ci j b) -> c ci b j", ci=CCH, j=2)
                nc.vector.tensor_add(dst, src, dst)

            for ci in range(CCH):
                nc.sync.dma_start(
                    out=y[:, ci * 128 : (ci + 1) * 128, l0 : l0 + LB].rearrange(
                        "b c l -> c b l"
                    ),
                    in_=xt_t[:, ci],
                )
    _hoist_extra_waits(nc)
    return nc


def _hoist_extra_waits(nc):
    """The 64B instruction encodings have room for only one embedded
    sem-wait, but Tile sometimes emits 2+ (foreign engine + self).  Splice
    same-engine NoOps (one wait each) before such instructions; the
    instruction keeps its last wait plus its sem updates."""
    import concourse.mybir as mybir

    n_fixed = 0
    for f in nc.m.functions:
        for blk in f.blocks:
            new_insts = []
            for inst in blk.instructions:
                si = inst.sync_info
                if si is not None and len(si.on_wait) > 1:
                    waits = list(si.on_wait)
                    for wi, w in enumerate(waits[:-1]):
                        nop = mybir.InstNoOp(
                            name=f"{inst.name}-wsp{wi}", ins=[], outs=[]
                        )
                        nop.engine = inst.engine
                        nop.sync_info = mybir.SyncInfo(on_wait=[w], on_update=[])
                        new_insts.append(nop)
                    inst.sync_info = mybir.SyncInfo(
                        on_wait=[waits[-1]], on_update=list(si.on_update)
                    )
                    n_fixed += 1
                new_insts.append(inst)
            if n_fixed:
                try:
                    blk.instructions = new_insts
                except Exception:
                    blk.instructions.clear()
                    blk.instructions.extend(new_insts)
    return n_fixed


_NC_CACHE = {}


def kernel(x: np.ndarray) -> np.ndarray:
    from concourse.bass_utils import run_bass_kernel_spmd

    assert x.shape == (B, C, L_FULL) and x.dtype == np.float32
    if "nc" not in _NC_CACHE:
        _NC_CACHE["nc"] = build_nc()
    nc = _NC_CACHE["nc"]

    in_maps = [
        {"x": np.ascontiguousarray(x[:, :, i * LS : (i + 1) * LS])}
        for i in range(N_CORES)
    ]
    res = run_bass_kernel_spmd(nc, in_maps, core_ids=list(range(N_CORES)))
    out = np.concatenate([res.results[i]["y"] for i in range(N_CORES)], axis=2)
    return out



# revision 23
# speedup vs baseline: 3.9143x; 3.9143x over previous
"""AttentionalPropagation kernel for Trainium2 (Bass/Tile), 8-core SPMD.

x: [B=64, C=512, L=4096] f32.  Per location l: self-attention over the B axis
(q=k=v, head dim C), out = x + msg.  Sharded over L: each of 8 cores handles
L/8 = 512 locations.

Host-side: per-core slice is transposed to location-major [LS*B, C] and cast
to bf16, so every DMA descriptor covers a full 1KB contiguous run (C row) --
that keeps the DMA engines at full modeled bandwidth (no sub-512B descriptor
penalty) and delivers q in [ (j,b), c ] layout, which is directly the rhs of
the second matmul (msg = E @ q) and the residual operand.

Per-core dataflow, 2 locations (one "pair") packed to fill the 128-wide PE,
4 pairs per "group", one group per pipeline slot:
  - DMA in: one group (4 pairs) -> q [128 (j,b), 4, 512] bf16
  - PE transpose (identity matmul) all 4 pairs into one [128,2048] bf16 psum,
    drained by ONE ACT copy -> qT bf16 (mm1 operands)
  - mm1: scores pair [128,128] psum = qT^T qT over 4 C-chunks (PE);
    4 pairs' scores packed into one [128,512] psum bank
  - exp(scale*scores) on the two diagonal 64x64 blocks of each pair, batched
    across 4 pairs per ACT op (strided AP); the rest of each rotating E tile
    is zeroed by gpsimd each round -> block-diagonal E bf16
  - rowsum per pair = E_blockdiag @ ones column (PE), written into the
    just-freed first 4 columns of the scores psum; one DVE reciprocal
  - mm2: msg pair [128,512] psum = E_blockdiag @ q (ONE K=128 matmul)
  - fused drain (DVE scalar_tensor_tensor): y = msg*inv + q -> bf16
  - DMA out (gpsimd SWDGE so its waits don't block the input-DMA queue)

The per-group stages are issued with an explicit software-pipeline skew
(stage S for group g issues in slot g+depth(S)) so that on every engine the
queue only contains work whose cross-engine producers completed in earlier
slots: no engine ever head-of-line blocks on a same-slot producer.
"""

import numpy as np

B, C, L_FULL, N_CORES = 64, 512, 4096, 8
LS = L_FULL // N_CORES   # 512 locations per core
GP = 4                   # pairs per group (pipeline slot unit)
N_PAIR = LS // 2         # 256 pairs per core
CCH = C // 128           # 4 c-chunks
SCALE = 1.0 / float(C) ** 0.5


def build_nc(ls=LS, hoist_waits=True, schedule=None):
    from contextlib import ExitStack

    import concourse.bass as bass
    import concourse.mybir as mybir
    from concourse.masks import make_identity
    from concourse.tile import TileContext

    f32 = mybir.dt.float32
    bf16 = mybir.dt.bfloat16
    AF = mybir.ActivationFunctionType
    ALU = mybir.AluOpType

    n_pair = ls // 2
    ng = n_pair // GP
    assert ng * GP == n_pair

    nc = bass.Bass()
    x = nc.dram_tensor("x", [ls * B, C], bf16, kind="ExternalInput")
    y = nc.dram_tensor("y", [ls * B, C], bf16, kind="ExternalOutput")

    if schedule is None:
        schedule = [
            ("in", 0), ("memset", 1), ("out", 5), ("exp", 3),
            ("transpose", 1), ("rowsum", 3), ("drain", 1), ("recip", 3),
            ("dmaout", 6), ("mm1", 2),
        ]
    sk = dict(schedule)
    with ExitStack() as ctx:
        tc = ctx.enter_context(TileContext(nc))
        const = ctx.enter_context(tc.tile_pool(name="const", bufs=1))
        q_pool = ctx.enter_context(tc.tile_pool(name="q", bufs=12))
        y_pool = ctx.enter_context(
            tc.tile_pool(name="y", bufs=5)
        )
        qt_pool = ctx.enter_context(tc.tile_pool(name="qt", bufs=4))
        e_pool = ctx.enter_context(tc.tile_pool(name="e", bufs=8))
        inv_pool = ctx.enter_context(
            tc.tile_pool(name="inv", bufs=sk["out"] - sk["recip"] + 2)
        )
        ps_qt_pool = ctx.enter_context(tc.tile_pool(name="ps_qt", bufs=2, space="PSUM"))
        ps_s_pool = ctx.enter_context(tc.tile_pool(name="ps_s", bufs=2, space="PSUM"))
        ps_m_pool = ctx.enter_context(tc.tile_pool(name="ps_m", bufs=2, space="PSUM"))

        ident = const.tile([128, 128], bf16)
        make_identity(nc, ident)
        ones = const.tile([128, 1], bf16)
        nc.gpsimd.memset(ones, 1.0)

        st = {}  # group -> dict of live tiles

        def stage_in(g):
            s = st[g] = {}
            q_t = s["q"] = q_pool.tile([128, GP, C], bf16, name="q", tag="q")
            r0 = g * GP * 128
            nc.sync.dma_start(
                out=q_t,
                in_=x[r0 : r0 + GP * 128, :].rearrange(
                    "(p j b) c -> (j b) p c", p=GP, j=2
                ),
            )

        def stage_memset(g):
            e4 = st[g]["e"] = e_pool.tile([128, GP * 128], bf16, name="e", tag="e")
            nc.gpsimd.memset(e4, 0.0)

        def stage_exp(g):
            s = st[g]
            e4v = s["e"].rearrange("q (g n) -> q g n", g=GP)
            s4v = s["s"].rearrange("q (g n) -> q g n", g=GP)
            nc.scalar.activation(
                e4v[0:64, :, 0:64], s4v[0:64, :, 0:64], AF.Exp, scale=SCALE
            )
            nc.scalar.activation(
                e4v[64:128, :, 64:128], s4v[64:128, :, 64:128], AF.Exp, scale=SCALE
            )

        def stage_transpose(g):
            s = st[g]
            ps_qt4 = s["ps_qt"] = ps_qt_pool.tile(
                [128, GP * 512], bf16, name="ps_qt"
            )
            for p in range(GP):
                for ci in range(CCH):
                    nc.tensor.transpose(
                        ps_qt4[:, p * 512 + ci * 128 : p * 512 + (ci + 1) * 128],
                        s["q"][:, p, ci * 128 : (ci + 1) * 128],
                        ident,
                    )

        def stage_rowsum(g):
            s = st[g]
            for i in range(GP):
                nc.tensor.matmul(
                    s["s"][:, i : i + 1],
                    s["e"][:, i * 128 : (i + 1) * 128],
                    ones,
                    start=True,
                    stop=True,
                )

        def stage_recip(g):
            s = st[g]
            inv4 = s["inv"] = inv_pool.tile([128, GP], f32, name="inv", tag="inv")
            nc.vector.reciprocal(inv4, s["s"][:, 0:GP])

        def stage_drain(g):
            s = st[g]
            qt4 = s["qt"] = qt_pool.tile([128, GP * 512], bf16, name="qt", tag="qt")
            nc.scalar.activation(qt4, s["ps_qt"], AF.Copy)

        def stage_out(g, pairs):
            s = st[g]
            if "y" not in s:
                s["y"] = y_pool.tile([128, GP, C], bf16, name="y", tag="y")
            y_t = s["y"]
            for i in pairs:
                ps_m = ps_m_pool.tile([128, 512], f32, name="ps_m")
                nc.tensor.matmul(
                    ps_m,
                    s["e"][:, i * 128 : (i + 1) * 128],
                    s["q"][:, i, :],
                    start=True,
                    stop=True,
                )
                nc.vector.scalar_tensor_tensor(
                    y_t[:, i, :],
                    ps_m,
                    s["inv"][:, i : i + 1],
                    s["q"][:, i, :],
                    ALU.mult,
                    ALU.add,
                )

        def stage_dma_out(g):
            s = st[g]
            r0 = g * GP * 128
            nc.gpsimd.dma_start(
                out=y[r0 : r0 + GP * 128, :].rearrange(
                    "(p j b) c -> (j b) p c", p=GP, j=2
                ),
                in_=s["y"],
            )
            del st[g]

        def stage_mm1(g):
            s = st[g]
            ps_s4 = s["s"] = ps_s_pool.tile([128, GP * 128], f32, name="ps_s")
            qt4 = s["qt"]
            for p in range(GP):
                for ci in range(CCH):
                    sl = qt4[:, p * 512 + ci * 128 : p * 512 + (ci + 1) * 128]
                    nc.tensor.matmul(
                        ps_s4[:, p * 128 : (p + 1) * 128],
                        sl,
                        sl,
                        start=(ci == 0),
                        stop=(ci == CCH - 1),
                    )

        # Software pipeline: stage S(g) issues in slot g + skew(S), in the
        # given per-slot issue order, so each engine's stream starts with
        # work whose producers finished in earlier slots.
        stages = {
            "in": stage_in,
            "memset": stage_memset,
            "out": lambda g: stage_out(g, range(GP)),
            "exp": stage_exp,
            "transpose": stage_transpose,
            "rowsum": stage_rowsum,
            "drain": stage_drain,
            "recip": stage_recip,
            "dmaout": stage_dma_out,
            "mm1": stage_mm1,
        }
        sched = schedule
        max_skew = max(k for _, k in sched)
        for t in range(ng + max_skew):
            for name, skew in sched:
                if 0 <= t - skew < ng:
                    stages[name](t - skew)
    if hoist_waits:
        _hoist_extra_waits(nc)
    return nc


def _hoist_extra_waits(nc):
    """The 64B instruction encodings have room for only one embedded
    sem-wait, but Tile sometimes emits 2+ (foreign engine + self).  Splice
    same-engine NoOps (one wait each) before such instructions; the
    instruction keeps its last wait plus its sem updates."""
    import concourse.mybir as mybir

    n_fixed = 0
    for f in nc.m.functions:
        for blk in f.blocks:
            new_insts = []
            for inst in blk.instructions:
                si = inst.sync_info
                if si is not None and len(si.on_wait) > 1:
                    waits = list(si.on_wait)
                    for wi, w in enumerate(waits[:-1]):
                        nop = mybir.InstNoOp(
                            name=f"{inst.name}-wsp{wi}", ins=[], outs=[]
                        )
                        nop.engine = inst.engine
                        nop.sync_info = mybir.SyncInfo(on_wait=[w], on_update=[])
                        new_insts.append(nop)
                    inst.sync_info = mybir.SyncInfo(
                        on_wait=[waits[-1]], on_update=list(si.on_update)
                    )
                    n_fixed += 1
                new_insts.append(inst)
            if n_fixed:
                try:
                    blk.instructions = new_insts
                except Exception:
                    blk.instructions.clear()
                    blk.instructions.extend(new_insts)
    return n_fixed


_NC_CACHE = {}


def kernel(x: np.ndarray) -> np.ndarray:
    import ml_dtypes

    from concourse.bass_utils import run_bass_kernel_spmd

    assert x.shape == (B, C, L_FULL) and x.dtype == np.float32
    if "nc" not in _NC_CACHE:
        _NC_CACHE["nc"] = build_nc()
    nc = _NC_CACHE["nc"]

    bf16 = ml_dtypes.bfloat16
    in_maps = []
    for i in range(N_CORES):
        xs = x[:, :, i * LS : (i + 1) * LS]              # [B, C, LS]
        xs = np.ascontiguousarray(xs.transpose(2, 0, 1))  # [LS, B, C]
        in_maps.append({"x": xs.reshape(LS * B, C).astype(bf16)})

    res = run_bass_kernel_spmd(nc, in_maps, core_ids=list(range(N_CORES)))
    outs = []
    for i in range(N_CORES):
        yi = np.asarray(res.results[i]["y"]).astype(np.float32)
        outs.append(yi.reshape(LS, B, C).transpose(1, 2, 0))  # [B, C, LS]
    return np.concatenate(outs, axis=2)


# revision 25
# speedup vs baseline: 3.9391x; 1.0063x over previous
"""AttentionalPropagation kernel for Trainium2 (Bass/Tile), 8-core SPMD.

x: [B=64, C=512, L=4096] f32.  Per location l: self-attention over the B axis
(q=k=v, head dim C), out = x + msg.  Sharded over L: each of 8 cores handles
L/8 = 512 locations.

Host-side: per-core slice is transposed to location-major [LS*B, C] and cast
to bf16, so every DMA descriptor covers a full 1KB contiguous run (C row) --
that keeps the DMA engines at full modeled bandwidth (no sub-512B descriptor
penalty) and delivers q in [ (j,b), c ] layout, which is directly the rhs of
the second matmul (msg = E @ q) and the residual operand.

Per-core dataflow, 2 locations (one "pair") packed to fill the 128-wide PE,
4 pairs per "group", one group per pipeline slot:
  - DMA in: one group (4 pairs) -> q [128 (j,b), 4, 512] bf16
  - PE transpose (identity matmul) all 4 pairs into one [128,2048] bf16 psum,
    drained by ONE ACT copy -> qT bf16 (mm1 operands)
  - mm1: scores pair [128,128] psum = qT^T qT over 4 C-chunks (PE);
    4 pairs' scores packed into one [128,512] psum bank
  - exp(scale*scores) on the two diagonal 64x64 blocks of each pair, batched
    across 4 pairs per ACT op (strided AP); the rest of each rotating E tile
    is zeroed by gpsimd each round -> block-diagonal E bf16
  - rowsum per pair = E_blockdiag @ ones column (PE), written into the
    just-freed first 4 columns of the scores psum; one DVE reciprocal
  - mm2: msg pair [128,512] psum = E_blockdiag @ q (ONE K=128 matmul)
  - fused drain (DVE scalar_tensor_tensor): y = msg*inv + q -> bf16
  - DMA out (gpsimd SWDGE so its waits don't block the input-DMA queue)

The per-group stages are issued with an explicit software-pipeline skew
(stage S for group g issues in slot g+skew(S)).  The skews and the tile-pool
buffer counts set the buffer-generation distances (and hence the inserted
semaphore dependencies): they are tuned so that every engine's queue only
contains work whose cross-engine producers completed in earlier slots, which
keeps the serialized DMA device >92% occupied -- the kernel is DMA-bound at
the bf16 in+out traffic floor (2 x 33.5 MB/core at 360 GB/s ~ 186 us).
"""

import numpy as np

B, C, L_FULL, N_CORES = 64, 512, 4096, 8
LS = L_FULL // N_CORES   # 512 locations per core
GP = 4                   # pairs per group (pipeline slot unit)
N_PAIR = LS // 2         # 256 pairs per core
CCH = C // 128           # 4 c-chunks
SCALE = 1.0 / float(C) ** 0.5


def build_nc(ls=LS, hoist_waits=True, schedule=None):
    from contextlib import ExitStack

    import concourse.bass as bass
    import concourse.mybir as mybir
    from concourse.masks import make_identity
    from concourse.tile import TileContext

    f32 = mybir.dt.float32
    bf16 = mybir.dt.bfloat16
    AF = mybir.ActivationFunctionType
    ALU = mybir.AluOpType

    n_pair = ls // 2
    ng = n_pair // GP
    assert ng * GP == n_pair

    nc = bass.Bass()
    x = nc.dram_tensor("x", [ls * B, C], bf16, kind="ExternalInput")
    y = nc.dram_tensor("y", [ls * B, C], bf16, kind="ExternalOutput")

    if schedule is None:
        schedule = [
            ("in", 0), ("memset", 1), ("out", 5), ("exp", 3),
            ("transpose", 1), ("rowsum", 3), ("drain", 1), ("recip", 3),
            ("dmaout", 6), ("mm1", 2),
        ]
    sk = dict(schedule)
    with ExitStack() as ctx:
        tc = ctx.enter_context(TileContext(nc))
        const = ctx.enter_context(tc.tile_pool(name="const", bufs=1))
        q_pool = ctx.enter_context(tc.tile_pool(name="q", bufs=12))
        y_pool = ctx.enter_context(
            tc.tile_pool(name="y", bufs=5)
        )
        qt_pool = ctx.enter_context(tc.tile_pool(name="qt", bufs=4))
        e_pool = ctx.enter_context(tc.tile_pool(name="e", bufs=8))
        inv_pool = ctx.enter_context(
            tc.tile_pool(name="inv", bufs=sk["out"] - sk["recip"] + 2)
        )
        ps_qt_pool = ctx.enter_context(tc.tile_pool(name="ps_qt", bufs=2, space="PSUM"))
        ps_s_pool = ctx.enter_context(tc.tile_pool(name="ps_s", bufs=2, space="PSUM"))
        ps_m_pool = ctx.enter_context(tc.tile_pool(name="ps_m", bufs=2, space="PSUM"))

        ident = const.tile([128, 128], bf16)
        make_identity(nc, ident)
        ones = const.tile([128, 1], bf16)
        nc.gpsimd.memset(ones, 1.0)

        st = {}  # group -> dict of live tiles

        def stage_in(g):
            s = st[g] = {}
            q_t = s["q"] = q_pool.tile([128, GP, C], bf16, name="q", tag="q")
            r0 = g * GP * 128
            half = GP // 2
            for hh in range(2):
                nc.sync.dma_start(
                    out=q_t[:, hh * half : (hh + 1) * half, :],
                    in_=x[
                        r0 + hh * half * 128 : r0 + (hh + 1) * half * 128, :
                    ].rearrange("(p j b) c -> (j b) p c", p=half, j=2),
                )

        def stage_memset(g):
            e4 = st[g]["e"] = e_pool.tile([128, GP * 128], bf16, name="e", tag="e")
            nc.gpsimd.memset(e4, 0.0)

        def stage_exp(g):
            s = st[g]
            e4v = s["e"].rearrange("q (g n) -> q g n", g=GP)
            s4v = s["s"].rearrange("q (g n) -> q g n", g=GP)
            nc.scalar.activation(
                e4v[0:64, :, 0:64], s4v[0:64, :, 0:64], AF.Exp, scale=SCALE
            )
            nc.scalar.activation(
                e4v[64:128, :, 64:128], s4v[64:128, :, 64:128], AF.Exp, scale=SCALE
            )

        def stage_transpose(g):
            s = st[g]
            ps_qt4 = s["ps_qt"] = ps_qt_pool.tile(
                [128, GP * 512], bf16, name="ps_qt"
            )
            for p in range(GP):
                for ci in range(CCH):
                    nc.tensor.transpose(
                        ps_qt4[:, p * 512 + ci * 128 : p * 512 + (ci + 1) * 128],
                        s["q"][:, p, ci * 128 : (ci + 1) * 128],
                        ident,
                    )

        def stage_rowsum(g):
            s = st[g]
            for i in range(GP):
                nc.tensor.matmul(
                    s["s"][:, i : i + 1],
                    s["e"][:, i * 128 : (i + 1) * 128],
                    ones,
                    start=True,
                    stop=True,
                )

        def stage_recip(g):
            s = st[g]
            inv4 = s["inv"] = inv_pool.tile([128, GP], f32, name="inv", tag="inv")
            nc.vector.reciprocal(inv4, s["s"][:, 0:GP])

        def stage_drain(g):
            s = st[g]
            qt4 = s["qt"] = qt_pool.tile([128, GP * 512], bf16, name="qt", tag="qt")
            nc.scalar.activation(qt4, s["ps_qt"], AF.Copy)

        def stage_out(g, pairs):
            s = st[g]
            if "y" not in s:
                s["y"] = y_pool.tile([128, GP, C], bf16, name="y", tag="y")
            y_t = s["y"]
            for i in pairs:
                ps_m = ps_m_pool.tile([128, 512], f32, name="ps_m")
                nc.tensor.matmul(
                    ps_m,
                    s["e"][:, i * 128 : (i + 1) * 128],
                    s["q"][:, i, :],
                    start=True,
                    stop=True,
                )
                nc.vector.scalar_tensor_tensor(
                    y_t[:, i, :],
                    ps_m,
                    s["inv"][:, i : i + 1],
                    s["q"][:, i, :],
                    ALU.mult,
                    ALU.add,
                )

        def stage_dma_out(g):
            s = st[g]
            r0 = g * GP * 128
            nc.gpsimd.dma_start(
                out=y[r0 : r0 + GP * 128, :].rearrange(
                    "(p j b) c -> (j b) p c", p=GP, j=2
                ),
                in_=s["y"],
            )
            del st[g]

        def stage_mm1(g):
            s = st[g]
            ps_s4 = s["s"] = ps_s_pool.tile([128, GP * 128], f32, name="ps_s")
            qt4 = s["qt"]
            for p in range(GP):
                for ci in range(CCH):
                    sl = qt4[:, p * 512 + ci * 128 : p * 512 + (ci + 1) * 128]
                    nc.tensor.matmul(
                        ps_s4[:, p * 128 : (p + 1) * 128],
                        sl,
                        sl,
                        start=(ci == 0),
                        stop=(ci == CCH - 1),
                    )

        # Software pipeline: stage S(g) issues in slot g + skew(S), in the
        # given per-slot issue order, so each engine's stream starts with
        # work whose producers finished in earlier slots.
        stages = {
            "in": stage_in,
            "memset": stage_memset,
            "out": lambda g: stage_out(g, range(GP)),
            "exp": stage_exp,
            "transpose": stage_transpose,
            "rowsum": stage_rowsum,
            "drain": stage_drain,
            "recip": stage_recip,
            "dmaout": stage_dma_out,
            "mm1": stage_mm1,
        }
        sched = schedule
        max_skew = max(k for _, k in sched)
        for t in range(ng + max_skew):
            for name, skew in sched:
                if 0 <= t - skew < ng:
                    stages[name](t - skew)
    if hoist_waits:
        _hoist_extra_waits(nc)
    return nc


def _hoist_extra_waits(nc):
    """The 64B instruction encodings have room for only one embedded
    sem-wait, but Tile sometimes emits 2+ (foreign engine + self).  Splice
    same-engine NoOps (one wait each) before such instructions; the
    instruction keeps its last wait plus its sem updates."""
    import concourse.mybir as mybir

    n_fixed = 0
    for f in nc.m.functions:
        for blk in f.blocks:
            new_insts = []
            for inst in blk.instructions:
                si = inst.sync_info
                if si is not None and len(si.on_wait) > 1:
                    waits = list(si.on_wait)
                    for wi, w in enumerate(waits[:-1]):
                        nop = mybir.InstNoOp(
                            name=f"{inst.name}-wsp{wi}", ins=[], outs=[]
                        )
                        nop.engine = inst.engine
                        nop.sync_info = mybir.SyncInfo(on_wait=[w], on_update=[])
                        new_insts.append(nop)
                    inst.sync_info = mybir.SyncInfo(
                        on_wait=[waits[-1]], on_update=list(si.on_update)
                    )
                    n_fixed += 1
                new_insts.append(inst)
            if n_fixed:
                try:
                    blk.instructions = new_insts
                except Exception:
                    blk.instructions.clear()
                    blk.instructions.extend(new_insts)
    return n_fixed


_NC_CACHE = {}


def kernel(x: np.ndarray) -> np.ndarray:
    import ml_dtypes

    from concourse.bass_utils import run_bass_kernel_spmd

    assert x.shape == (B, C, L_FULL) and x.dtype == np.float32
    if "nc" not in _NC_CACHE:
        _NC_CACHE["nc"] = build_nc()
    nc = _NC_CACHE["nc"]

    bf16 = ml_dtypes.bfloat16
    in_maps = []
    for i in range(N_CORES):
        xs = x[:, :, i * LS : (i + 1) * LS]              # [B, C, LS]
        xs = np.ascontiguousarray(xs.transpose(2, 0, 1))  # [LS, B, C]
        in_maps.append({"x": xs.reshape(LS * B, C).astype(bf16)})

    res = run_bass_kernel_spmd(nc, in_maps, core_ids=list(range(N_CORES)))
    outs = []
    for i in range(N_CORES):
        yi = np.asarray(res.results[i]["y"]).astype(np.float32)
        outs.append(yi.reshape(LS, B, C).transpose(1, 2, 0))  # [B, C, LS]
    return np.concatenate(outs, axis=2)

